# revision 1
# baseline (speedup 1.0000x reference)
"""Trainium2 Bass kernel for nn_DialogActLabeller (segment_reduce).

Computes, for input enc_output [32, 4096, 1024], W [1024, 256], b [256],
cls_pos [32, 64], last_sep [32]:

    x = enc_output @ W + b                      # [B, S, 256]
    seg[b, n] = sum_{s in [start_n, end_n)} x[b, s, :]
    out = log_softmax(seg, axis=-1)             # [B, 64, 256]

Key algebraic restructure: the projection is linear, so segment-reduce
FIRST on enc_output (via a matmul with a 0/1 segment-indicator matrix A),
then project the tiny [64, 1024] per-batch result with W, and add
len_n * b for the bias.  This reads enc_output exactly once from HBM and
does ~1/32 of the naive FLOPs.

Sharding: pure data parallel, 4 batch rows per core across 8 cores
(W, b replicated), no cross-core communication.
"""

import os
import numpy as np

import concourse.bacc as bacc
import concourse.bass as bass
import concourse.tile as tile
from concourse import mybir
from concourse import bass_utils
from contextlib import ExitStack

# Problem shapes (hardcoded per contract)
B, S, D_IN, D_OUT, N_SENT = 32, 4096, 1024, 256, 64
N_CORES = 8
BPC = B // N_CORES          # batches per core
SCHUNKS = S // 128          # 32 sequence chunks of 128
DCH = D_IN // 128           # 8 d_in chunks of 128
SS_PER_DMA = 8              # s-chunks per enc DMA (4 MiB transfers)

F32 = mybir.dt.float32

# Matmul dtype for the big segment-reduce matmul: float32r streams 4x faster
# through the PE than float32 on TRN2 (fp32 bits, reduced-precision multiply).
# The small projection matmul stays plain float32.
_SEG_MM_DT = getattr(mybir.dt, os.environ.get("SEG_MM_DT", "float32r"))


def _build_program():
    nc = bacc.Bacc("TRN2", debug=False)

    # The segment-reduce matmul operands are declared end-to-end in the
    # matmul dtype (float32r is bit-identical to float32 in memory, so the
    # host still feeds plain fp32 arrays and the DMA is a plain copy).
    #
    # enc is host-pre-tiled to [BPC, n_dma, 128, SS_PER_DMA*D_IN] so each DMA
    # reads one fully-contiguous 32 KiB run per partition (minimal descriptors).
    n_dma = SCHUNKS // SS_PER_DMA
    enc = nc.dram_tensor(
        "enc", [BPC, n_dma, 128, SS_PER_DMA * D_IN], _SEG_MM_DT, kind="ExternalInput"
    ).ap()
    # W host-pre-tiled to [128, DCH*D_OUT] with layout [p, j, o]
    wt = nc.dram_tensor("w", [128, DCH * D_OUT], F32, kind="ExternalInput").ap()
    bias = nc.dram_tensor("bias", [D_OUT], F32, kind="ExternalInput").ap()
    amat = nc.dram_tensor(
        "amat", [BPC, 128, SCHUNKS * N_SENT], mybir.dt.uint8, kind="ExternalInput"
    ).ap()
    lens = nc.dram_tensor("lens", [BPC, N_SENT], F32, kind="ExternalInput").ap()
    ident = nc.dram_tensor("ident", [128, 128], F32, kind="ExternalInput").ap()
    out = nc.dram_tensor(
        "out", [BPC, N_SENT, D_OUT], F32, kind="ExternalOutput"
    ).ap()

    with tile.TileContext(nc) as tc, ExitStack() as ctx:
        singles = ctx.enter_context(tc.tile_pool(name="singles", bufs=1))
        encp = ctx.enter_context(tc.tile_pool(name="encp", bufs=4))
        apool = ctx.enter_context(tc.tile_pool(name="apool", bufs=2))
        segp = ctx.enter_context(tc.tile_pool(name="segp", bufs=2))
        smalls = ctx.enter_context(tc.tile_pool(name="smalls", bufs=4))
        ps_seg = ctx.enter_context(tc.tile_pool(name="ps_seg", bufs=2, space="PSUM"))
        ps_tr = ctx.enter_context(tc.tile_pool(name="ps_tr", bufs=2, space="PSUM"))
        ps_pr = ctx.enter_context(tc.tile_pool(name="ps_pr", bufs=2, space="PSUM"))

        # ---- constants, loaded once (issued on the ACT HWDGE ring so they
        # don't delay the enc stream on the Sync ring) ----
        w_sb = singles.tile([128, DCH, D_OUT], F32)
        nc.scalar.dma_start(out=w_sb, in_=wt.rearrange("p (j o) -> p j o", o=D_OUT))
        ident_sb = singles.tile([128, 128], F32)
        nc.scalar.dma_start(out=ident_sb, in_=ident)
        # b broadcast to [N_SENT, D_OUT] via stride-0 partition AP (SWDGE)
        b_bc = singles.tile([N_SENT, D_OUT], F32)
        bias_bcast = bass.AP(
            tensor=bias.tensor, offset=bias.offset,
            ap=[[0, N_SENT], [1, D_OUT]],
        )
        nc.gpsimd.dma_start(out=b_bc, in_=bias_bcast)
        # lens transposed into [N_SENT, BPC] so lens[:, bi] is a per-partition scalar
        lens_sb = singles.tile([N_SENT, BPC], F32)
        nc.scalar.dma_start(out=lens_sb, in_=lens.rearrange("b n -> n b"))

        # all-batch softmax staging tiles
        sv_all = singles.tile([N_SENT, BPC, D_OUT], F32)
        svs_all = singles.tile([N_SENT, BPC, D_OUT], F32)

        # all batches' segment-indicator matrices, shipped as uint8 in one DMA
        a_u8 = singles.tile([128, BPC, SCHUNKS * N_SENT], mybir.dt.uint8)
        nc.scalar.dma_start(
            out=a_u8, in_=amat.rearrange("b p kn -> p b kn")
        )

        for bi in range(BPC):
            # expand this batch's indicator matrix to the matmul dtype on the DVE
            a_sb = apool.tile([128, SCHUNKS, N_SENT], _SEG_MM_DT, tag="a")
            nc.vector.tensor_copy(
                out=a_sb, in_=a_u8[:, bi].rearrange("p (k n) -> p k n", n=N_SENT)
            )

            # ---- segment reduce: seg[n, d] = sum_s A[s, n] * enc[s, d] ----
            ps0 = ps_seg.tile([N_SENT, 512], F32, tag="ps0")
            ps1 = ps_seg.tile([N_SENT, 512], F32, tag="ps1")
            for kk in range(n_dma):
                et = encp.tile([128, SS_PER_DMA, D_IN], _SEG_MM_DT, tag="enc")
                nc.sync.dma_start(
                    out=et,
                    in_=enc[bi, kk].rearrange("p (t d) -> p t d", d=D_IN),
                )
                for t in range(SS_PER_DMA):
                    k = kk * SS_PER_DMA + t
                    lhsT = a_sb[:, k, :]
                    for dh in range(2):
                        rhs = et[:, t, dh * 512 : (dh + 1) * 512]
                        nc.tensor.matmul(
                            ps0 if dh == 0 else ps1,
                            lhsT=lhsT,
                            rhs=rhs,
                            start=(k == 0),
                            stop=(k == SCHUNKS - 1),
                        )

            seg_sb = segp.tile([N_SENT, D_IN], F32, tag="seg")
            nc.vector.tensor_copy(out=seg_sb[:, 0:512], in_=ps0)
            nc.vector.tensor_copy(out=seg_sb[:, 512:1024], in_=ps1)

            # ---- transpose seg [64, 1024] -> segT [128(d), 8(j), 64(n)] ----
            seg_t = segp.tile([128, DCH, N_SENT], F32, tag="segT")
            for j in range(DCH):
                pt = ps_tr.tile([128, N_SENT], F32, tag="pt")
                nc.tensor.transpose(
                    out=pt,
                    in_=seg_sb[:, j * 128 : (j + 1) * 128],
                    identity=ident_sb[0:N_SENT, 0:N_SENT],
                )
                nc.vector.tensor_copy(out=seg_t[:, j, :], in_=pt)

            # ---- projection: sv[n, o] = sum_d segT[d, n] * W[d, o] ----
            pp = ps_pr.tile([N_SENT, D_OUT], F32, tag="pp")
            for j in range(DCH):
                nc.tensor.matmul(
                    pp,
                    lhsT=seg_t[:, j, :],
                    rhs=w_sb[:, j, :],
                    start=(j == 0),
                    stop=(j == DCH - 1),
                )

            # ---- sv = pp + len * b, staged into the all-batch tile ----
            nc.vector.scalar_tensor_tensor(
                out=sv_all[:, bi, :],
                in0=b_bc,
                scalar=lens_sb[:, bi : bi + 1],
                in1=pp,
                op0=mybir.AluOpType.mult,
                op1=mybir.AluOpType.add,
            )
            # per-batch shifted logits: svs = sv - max(sv)
            negmax = smalls.tile([N_SENT, 1], F32, tag=f"negmax{bi}", bufs=1)
            nc.vector.tensor_reduce(
                out=negmax, in_=sv_all[:, bi, :], axis=mybir.AxisListType.X,
                op=mybir.AluOpType.max, negate=True,
            )
            nc.vector.tensor_scalar(
                out=svs_all[:, bi, :], in0=sv_all[:, bi, :], scalar1=negmax,
                scalar2=None, op0=mybir.AluOpType.add,
            )

        # ---- batched log_softmax tail: one Exp + one Ln for all batches ----
        ex_all = singles.tile([N_SENT, BPC, D_OUT], F32)
        nc.scalar.activation(
            out=ex_all, in_=svs_all, func=mybir.ActivationFunctionType.Exp,
        )
        ssum_all = smalls.tile([N_SENT, BPC], F32, tag="ssum", bufs=1)
        nc.vector.tensor_reduce(
            out=ssum_all, in_=ex_all, axis=mybir.AxisListType.X,
            op=mybir.AluOpType.add,
        )
        lse_all = smalls.tile([N_SENT, BPC], F32, tag="lse", bufs=1)
        nc.scalar.activation(
            out=lse_all, in_=ssum_all, func=mybir.ActivationFunctionType.Ln
        )
        ot_all = singles.tile([N_SENT, BPC, D_OUT], F32)
        for bi in range(BPC):
            nc.vector.tensor_scalar(
                out=ot_all[:, bi, :], in0=svs_all[:, bi, :],
                scalar1=lse_all[:, bi : bi + 1], scalar2=None,
                op0=mybir.AluOpType.subtract,
            )
        nc.sync.dma_start(out=out.rearrange("b n o -> n b o"), in_=ot_all)

    nc.compile()
    return nc


_PROGRAM = None


def _get_program():
    global _PROGRAM
    if _PROGRAM is None:
        _PROGRAM = _build_program()
    return _PROGRAM


def _host_prep(enc_output, W, b, cls_pos, last_sep):
    n_dma = SCHUNKS // SS_PER_DMA
    enc = np.asarray(enc_output, dtype=np.float32)
    # pre-tile so each DMA reads one contiguous 32 KiB run per partition:
    # [B, S, D] -> [B, n_dma, 128(p), SS_PER_DMA(t) * D]  with s = (kk*SS+t)*128+p
    enc = np.ascontiguousarray(
        enc.reshape(B, n_dma, SS_PER_DMA, 128, D_IN)
        .transpose(0, 1, 3, 2, 4)
        .reshape(B, n_dma, 128, SS_PER_DMA * D_IN)
    )
    wf = np.asarray(W, dtype=np.float32)
    # [D_IN, D_OUT] -> [128(p), DCH(j) * D_OUT] with d = j*128+p
    wf = np.ascontiguousarray(
        wf.reshape(DCH, 128, D_OUT).transpose(1, 0, 2).reshape(128, DCH * D_OUT)
    )
    bf = np.ascontiguousarray(np.asarray(b, dtype=np.float32))
    starts = np.asarray(cls_pos).astype(np.int64)                    # [B, N]
    lsep = np.asarray(last_sep).astype(np.int64)                     # [B]
    ends = np.concatenate([starts[:, 1:], (lsep + 1)[:, None]], axis=1)
    # torch semantics for the last segment: if end <= start, sum to seq end
    ends[:, -1] = np.where(ends[:, -1] > starts[:, -1], ends[:, -1], S)
    lens = (ends - starts).astype(np.float32)                        # [B, N]

    s = np.arange(S, dtype=np.int64)
    afull = (s[None, :, None] >= starts[:, None, :]) & (
        s[None, :, None] < ends[:, None, :]
    )                                                                # [B, S, N]
    amat = (
        afull.reshape(B, SCHUNKS, 128, N_SENT)
        .transpose(0, 2, 1, 3)
        .reshape(B, 128, SCHUNKS * N_SENT)
        .astype(np.uint8)
    )
    return enc, wf, bf, amat, lens


def kernel(enc_output, W, b, max_num_sent, cls_pos, last_sep, _trace=False):
    enc, wf, bf, amat, lens = _host_prep(enc_output, W, b, cls_pos, last_sep)
    ident = np.eye(128, dtype=np.float32)

    nc = _get_program()
    in_maps = []
    for c in range(N_CORES):
        bsl = slice(c * BPC, (c + 1) * BPC)
        in_maps.append(
            {
                "enc": enc[bsl],
                "w": wf,
                "bias": bf,
                "amat": amat[bsl],
                "lens": lens[bsl],
                "ident": ident,
            }
        )
    res = bass_utils.run_bass_kernel_spmd(
        nc, in_maps, core_ids=list(range(N_CORES)), trace=_trace
    )
    out = np.concatenate(
        [res.results[c]["out"][None] for c in range(N_CORES)], axis=0
    ).reshape(B, N_SENT, D_OUT)
    if _trace:
        kernel._last_result = res
    return out.astype(np.float32)



# revision 4
# speedup vs baseline: 2.4910x; 2.4910x over previous
"""Trainium2 Bass kernel for nn_DialogActLabeller (segment_reduce).

Computes, for input enc_output [32, 4096, 1024], W [1024, 256], b [256],
cls_pos [32, 64], last_sep [32]:

    x = enc_output @ W + b                      # [B, S, 256]
    seg[b, n] = sum_{s in [start_n, end_n)} x[b, s, :]
    out = log_softmax(seg, axis=-1)             # [B, 64, 256]

Key algebraic restructure: the projection is linear, so segment-reduce
FIRST on enc_output (via a matmul with a 0/1 segment-indicator matrix A),
then project the tiny [64, 1024] per-batch result with W, and add
len_n * b for the bias.  This reads enc_output exactly once from HBM and
does ~1/32 of the naive FLOPs.

The kernel is HBM-bound (enc_output is 512 MiB), so enc is shipped as
fp8 e4m3.  Plain per-element rounding would make segment sums drift as
sqrt(len); instead the host quantizes with error diffusion along s
(within each 128-position block): the running rounding error is carried
into the next element, so partial sums telescope and each segment sum
carries only ~one rounding step of error regardless of length.

The segment-reduce matmul then runs in fp8 DoubleRow mode (two stacked
128x64 weight sets -> full PE array, 2 contraction rows per cycle).

Sharding: pure data parallel, 4 batch rows per core across 8 cores
(W, b replicated), no cross-core communication.
"""

import numpy as np

import concourse.bacc as bacc
import concourse.bass as bass
import concourse.tile as tile
from concourse import mybir
from concourse import bass_utils
from contextlib import ExitStack

# Problem shapes (hardcoded per contract)
B, S, D_IN, D_OUT, N_SENT = 32, 4096, 1024, 256, 64
N_CORES = 8
BPC = B // N_CORES          # batches per core
SCHUNKS = S // 128          # 32 sequence chunks of 128
DCH = D_IN // 128           # 8 d_in chunks of 128
SS_PER_DMA = 8              # s-chunks per enc DMA (1 MiB fp8 transfers)

F32 = mybir.dt.float32
F32R = mybir.dt.float32r
FP8 = mybir.dt.float8e4
_E4NP = mybir.dt.np(FP8)    # ml_dtypes.float8_e4m3


def _build_program():
    nc = bacc.Bacc("TRN2", debug=False)

    # enc is host-pre-tiled to [BPC, n_dma, 128, SS_PER_DMA*D_IN] so each DMA
    # reads one fully-contiguous 8 KiB run per partition (minimal descriptors).
    n_dma = SCHUNKS // SS_PER_DMA
    enc = nc.dram_tensor(
        "enc", [BPC, n_dma, 128, SS_PER_DMA * D_IN], FP8, kind="ExternalInput"
    ).ap()
    # W host-pre-tiled to [128, DCH*D_OUT] with layout [p, j, o]
    wt = nc.dram_tensor("w", [128, DCH * D_OUT], F32R, kind="ExternalInput").ap()
    bias = nc.dram_tensor("bias", [D_OUT], F32, kind="ExternalInput").ap()
    # segment-indicator matrices, already in fp8 (0.0 / 1.0 bytes)
    amat = nc.dram_tensor(
        "amat", [BPC, 128, SCHUNKS * N_SENT], FP8, kind="ExternalInput"
    ).ap()
    lens = nc.dram_tensor("lens", [BPC, N_SENT], F32, kind="ExternalInput").ap()
    ident = nc.dram_tensor("ident", [128, 128], F32, kind="ExternalInput").ap()
    out = nc.dram_tensor(
        "out", [BPC, N_SENT, D_OUT], F32, kind="ExternalOutput"
    ).ap()

    with tile.TileContext(nc) as tc, ExitStack() as ctx:
        singles = ctx.enter_context(tc.tile_pool(name="singles", bufs=1))
        encp = ctx.enter_context(tc.tile_pool(name="encp", bufs=4))
        segp = ctx.enter_context(tc.tile_pool(name="segp", bufs=2))
        smalls = ctx.enter_context(tc.tile_pool(name="smalls", bufs=4))
        ps_seg = ctx.enter_context(tc.tile_pool(name="ps_seg", bufs=2, space="PSUM"))
        ps_tr = ctx.enter_context(tc.tile_pool(name="ps_tr", bufs=2, space="PSUM"))
        ps_pr = ctx.enter_context(tc.tile_pool(name="ps_pr", bufs=2, space="PSUM"))

        # ---- constants, loaded once on the ACT HWDGE ring so they don't
        # delay the enc stream on the Sync ring.  amat first: the first
        # matmul needs it. ----
        a_sb = singles.tile([128, BPC, SCHUNKS, N_SENT], FP8)
        nc.scalar.dma_start(
            out=a_sb,
            in_=amat.rearrange("b p (k n) -> p b k n", n=N_SENT),
        )
        ident_sb = singles.tile([128, 128], F32)
        nc.scalar.dma_start(out=ident_sb, in_=ident)
        # lens transposed into [N_SENT, BPC] so lens[:, bi] is a per-partition scalar
        lens_sb = singles.tile([N_SENT, BPC], F32)
        nc.scalar.dma_start(out=lens_sb, in_=lens.rearrange("b n -> n b"))
        w_sb = singles.tile([128, DCH, D_OUT], F32R)
        nc.scalar.dma_start(out=w_sb, in_=wt.rearrange("p (j o) -> p j o", o=D_OUT))
        # b broadcast to [N_SENT, D_OUT] via stride-0 partition AP (SWDGE)
        b_bc = singles.tile([N_SENT, D_OUT], F32)
        bias_bcast = bass.AP(
            tensor=bias.tensor, offset=bias.offset,
            ap=[[0, N_SENT], [1, D_OUT]],
        )
        nc.gpsimd.dma_start(out=b_bc, in_=bias_bcast)

        # all-batch softmax staging tiles
        sv_all = singles.tile([N_SENT, BPC, D_OUT], F32)
        svs_all = singles.tile([N_SENT, BPC, D_OUT], F32)

        n_pairs = SCHUNKS // 2
        pairs_per_dma = SS_PER_DMA // 2
        for bi in range(BPC):
            # ---- segment reduce: seg[n, d] = sum_s A[s, n] * enc[s, d] ----
            # fp8 DoubleRow: lhsT [128, 2, 64] (two stacked weight sets),
            # rhs [128, 2, 512], out [64, 512]; 2 contraction rows/cycle.
            ps0 = ps_seg.tile([N_SENT, 512], F32, tag="ps0")
            ps1 = ps_seg.tile([N_SENT, 512], F32, tag="ps1")
            for kk in range(n_dma):
                et = encp.tile([128, SS_PER_DMA, D_IN], FP8, tag="enc")
                nc.sync.dma_start(
                    out=et,
                    in_=enc[bi, kk].rearrange("p (t d) -> p t d", d=D_IN),
                )
                for u in range(pairs_per_dma):
                    pair = kk * pairs_per_dma + u
                    lhsT = a_sb[:, bi, 2 * pair : 2 * pair + 2, :]
                    for dh in range(2):
                        rhs = et[:, 2 * u : 2 * u + 2, dh * 512 : (dh + 1) * 512]
                        nc.tensor.matmul(
                            ps0 if dh == 0 else ps1,
                            lhsT=lhsT,
                            rhs=rhs,
                            start=(pair == 0),
                            stop=(pair == n_pairs - 1),
                            perf_mode=mybir.MatmulPerfMode.DoubleRow,
                        )

            seg_sb = segp.tile([N_SENT, D_IN], F32, tag="seg")
            nc.vector.tensor_copy(out=seg_sb[:, 0:512], in_=ps0)
            nc.scalar.copy(out=seg_sb[:, 512:1024], in_=ps1)

            # ---- transpose seg [64, 1024] -> segT [128(d), 8(j), 64(n)] ----
            seg_t = segp.tile([128, DCH, N_SENT], F32R, tag="segT")
            for j in range(DCH):
                pt = ps_tr.tile([128, N_SENT], F32, tag="pt")
                nc.tensor.transpose(
                    out=pt,
                    in_=seg_sb[:, j * 128 : (j + 1) * 128],
                    identity=ident_sb[0:N_SENT, 0:N_SENT],
                )
                if j % 2 == 0:
                    nc.vector.tensor_copy(out=seg_t[:, j, :], in_=pt)
                else:
                    nc.scalar.copy(out=seg_t[:, j, :], in_=pt)

            # ---- projection: sv[n, o] = sum_d segT[d, n] * W[d, o] ----
            # float32r streams 1 row/cycle at free size >= 256 (vs 4 for fp32)
            pp = ps_pr.tile([N_SENT, D_OUT], F32, tag="pp")
            for j in range(DCH):
                nc.tensor.matmul(
                    pp,
                    lhsT=seg_t[:, j, :],
                    rhs=w_sb[:, j, :],
                    start=(j == 0),
                    stop=(j == DCH - 1),
                )

            # ---- sv = pp + len * b, staged into the all-batch tile ----
            nc.vector.scalar_tensor_tensor(
                out=sv_all[:, bi, :],
                in0=b_bc,
                scalar=lens_sb[:, bi : bi + 1],
                in1=pp,
                op0=mybir.AluOpType.mult,
                op1=mybir.AluOpType.add,
            )
            # per-batch shifted logits: svs = sv - max(sv)
            negmax = smalls.tile([N_SENT, 1], F32, tag=f"negmax{bi}", bufs=1)
            nc.vector.tensor_reduce(
                out=negmax, in_=sv_all[:, bi, :], axis=mybir.AxisListType.X,
                op=mybir.AluOpType.max, negate=True,
            )
            nc.vector.tensor_scalar(
                out=svs_all[:, bi, :], in0=sv_all[:, bi, :], scalar1=negmax,
                scalar2=None, op0=mybir.AluOpType.add,
            )

        # ---- batched log_softmax tail: one Exp + one Ln for all batches ----
        ex_all = singles.tile([N_SENT, BPC, D_OUT], F32)
        nc.scalar.activation(
            out=ex_all, in_=svs_all, func=mybir.ActivationFunctionType.Exp,
        )
        ssum_all = smalls.tile([N_SENT, BPC], F32, tag="ssum", bufs=1)
        nc.vector.tensor_reduce(
            out=ssum_all, in_=ex_all, axis=mybir.AxisListType.X,
            op=mybir.AluOpType.add,
        )
        lse_all = smalls.tile([N_SENT, BPC], F32, tag="lse", bufs=1)
        nc.scalar.activation(
            out=lse_all, in_=ssum_all, func=mybir.ActivationFunctionType.Ln
        )
        ot_all = singles.tile([N_SENT, BPC, D_OUT], F32)
        for bi in range(BPC):
            nc.vector.tensor_scalar(
                out=ot_all[:, bi, :], in0=svs_all[:, bi, :],
                scalar1=lse_all[:, bi : bi + 1], scalar2=None,
                op0=mybir.AluOpType.subtract,
            )
        nc.sync.dma_start(out=out.rearrange("b n o -> n b o"), in_=ot_all)

    nc.compile()
    return nc


_PROGRAM = None


def _get_program():
    global _PROGRAM
    if _PROGRAM is None:
        _PROGRAM = _build_program()
    return _PROGRAM


def _quantize_diffuse(enc):
    """fp8 e4m3 quantization with error diffusion along s (block=128).

    Within each contiguous 128-position block the rounding error of each
    element is carried into the next, so any in-block partial sum of the
    quantized values equals the exact partial sum plus at most ~one
    rounding step.  Segment sums then see only ~one step of error per
    block boundary crossed instead of sqrt(len) growth.
    """
    enc_r = enc.reshape(B, SCHUNKS, 128, D_IN)
    q = np.empty((B, SCHUNKS, 128, D_IN), dtype=_E4NP)
    carry = np.zeros((B, SCHUNKS, D_IN), dtype=np.float32)
    for i in range(128):
        t = enc_r[:, :, i, :] + carry
        qi = t.astype(_E4NP)
        q[:, :, i, :] = qi
        carry = t - qi.astype(np.float32)
    return q  # [B, k, p, D] with s = k*128 + p


def _host_prep(enc_output, W, b, cls_pos, last_sep):
    n_dma = SCHUNKS // SS_PER_DMA
    enc = np.asarray(enc_output, dtype=np.float32)
    q = _quantize_diffuse(enc)
    # [B, k, p, D] -> [B, n_dma, 128(p), SS_PER_DMA(t) * D]  with k = kk*SS+t
    enc8 = np.ascontiguousarray(
        q.reshape(B, n_dma, SS_PER_DMA, 128, D_IN)
        .transpose(0, 1, 3, 2, 4)
        .reshape(B, n_dma, 128, SS_PER_DMA * D_IN)
    )
    wf = np.asarray(W, dtype=np.float32)
    # [D_IN, D_OUT] -> [128(p), DCH(j) * D_OUT] with d = j*128+p
    wf = np.ascontiguousarray(
        wf.reshape(DCH, 128, D_OUT).transpose(1, 0, 2).reshape(128, DCH * D_OUT)
    )
    bf = np.ascontiguousarray(np.asarray(b, dtype=np.float32))
    starts = np.asarray(cls_pos).astype(np.int64)                    # [B, N]
    lsep = np.asarray(last_sep).astype(np.int64)                     # [B]
    ends = np.concatenate([starts[:, 1:], (lsep + 1)[:, None]], axis=1)
    # torch semantics for the last segment: if end <= start, sum to seq end
    ends[:, -1] = np.where(ends[:, -1] > starts[:, -1], ends[:, -1], S)
    lens = (ends - starts).astype(np.float32)                        # [B, N]

    s = np.arange(S, dtype=np.int64)
    afull = (s[None, :, None] >= starts[:, None, :]) & (
        s[None, :, None] < ends[:, None, :]
    )                                                                # [B, S, N]
    amat = (
        afull.reshape(B, SCHUNKS, 128, N_SENT)
        .transpose(0, 2, 1, 3)
        .reshape(B, 128, SCHUNKS * N_SENT)
        .astype(np.float32)
        .astype(_E4NP)                                               # 0.0 / 1.0 exact
    )
    return enc8, wf, bf, amat, lens


def kernel(enc_output, W, b, max_num_sent, cls_pos, last_sep, _trace=False):
    enc8, wf, bf, amat, lens = _host_prep(enc_output, W, b, cls_pos, last_sep)
    ident = np.eye(128, dtype=np.float32)

    nc = _get_program()
    in_maps = []
    for c in range(N_CORES):
        bsl = slice(c * BPC, (c + 1) * BPC)
        in_maps.append(
            {
                "enc": enc8[bsl],
                "w": wf,
                "bias": bf,
                "amat": amat[bsl],
                "lens": lens[bsl],
                "ident": ident,
            }
        )
    res = bass_utils.run_bass_kernel_spmd(
        nc, in_maps, core_ids=list(range(N_CORES)), trace=_trace
    )
    out = np.concatenate(
        [res.results[c]["out"][None] for c in range(N_CORES)], axis=0
    ).reshape(B, N_SENT, D_OUT)
    if _trace:
        kernel._last_result = res
    return out.astype(np.float32)


# revision 6
# speedup vs baseline: 2.5910x; 1.0401x over previous
"""Trainium2 Bass kernel for nn_DialogActLabeller (segment_reduce).

Computes, for input enc_output [32, 4096, 1024], W [1024, 256], b [256],
cls_pos [32, 64], last_sep [32]:

    x = enc_output @ W + b                      # [B, S, 256]
    seg[b, n] = sum_{s in [start_n, end_n)} x[b, s, :]
    out = log_softmax(seg, axis=-1)             # [B, 64, 256]

Key algebraic restructure: the projection is linear, so segment-reduce
FIRST on enc_output (via a matmul with a 0/1 segment-indicator matrix A),
then project the tiny [64, 1024] per-batch result with W, and add
len_n * b for the bias.  This reads enc_output exactly once from HBM and
does ~1/32 of the naive FLOPs.

The kernel is HBM-bound (enc_output is 512 MiB), so enc is shipped as
fp8 e4m3.  Plain per-element rounding would make segment sums drift as
sqrt(len); instead the host quantizes with error diffusion along s
(within each 128-position block): the running rounding error is carried
into the next element, so partial sums telescope and each segment sum
carries only ~one rounding step of error regardless of length.

The segment-reduce matmul runs in fp8 DoubleRow mode (two stacked
128x64 weight sets -> full PE array, 2 contraction rows per cycle).
The per-batch tail (PSUM evict, transpose, projection, softmax prep) is
software-pipelined one batch behind the enc stream so the PE queue never
stalls at batch boundaries.

Sharding: pure data parallel, 4 batch rows per core across 8 cores
(W, b replicated), no cross-core communication.
"""

import numpy as np

import concourse.bacc as bacc
import concourse.bass as bass
import concourse.tile as tile
from concourse import mybir
from concourse import bass_utils
from contextlib import ExitStack

# Problem shapes (hardcoded per contract)
B, S, D_IN, D_OUT, N_SENT = 32, 4096, 1024, 256, 64
N_CORES = 8
BPC = B // N_CORES          # batches per core
SCHUNKS = S // 128          # 32 sequence chunks of 128
DCH = D_IN // 128           # 8 d_in chunks of 128
SS_PER_DMA = 8              # s-chunks per enc DMA (1 MiB fp8 transfers)

F32 = mybir.dt.float32
BF16 = mybir.dt.bfloat16
FP8 = mybir.dt.float8e4
_E4NP = mybir.dt.np(FP8)    # ml_dtypes.float8_e4m3
_BF16NP = mybir.dt.np(BF16)


def _build_program():
    nc = bacc.Bacc("TRN2", debug=False)

    n_dma = SCHUNKS // SS_PER_DMA
    enc = nc.dram_tensor(
        "enc", [BPC, n_dma, 128, SS_PER_DMA * D_IN], FP8, kind="ExternalInput"
    ).ap()
    # W host-pre-tiled to bf16 [128, DCH*D_OUT] with layout [p, j, o]
    wt = nc.dram_tensor("w", [128, DCH * D_OUT], BF16, kind="ExternalInput").ap()
    bias = nc.dram_tensor("bias", [D_OUT], F32, kind="ExternalInput").ap()
    # segment-indicator matrices in fp8, host-pre-tiled to the exact SBUF
    # layout [128(p), BPC, SCHUNKS, N_SENT] so the DMA is fully contiguous
    amat = nc.dram_tensor(
        "amat", [128, BPC * SCHUNKS * N_SENT], FP8, kind="ExternalInput"
    ).ap()
    lens = nc.dram_tensor("lens", [N_SENT, BPC], F32, kind="ExternalInput").ap()
    ident = nc.dram_tensor("ident", [128, 128], BF16, kind="ExternalInput").ap()
    out = nc.dram_tensor(
        "out", [BPC, N_SENT, D_OUT], F32, kind="ExternalOutput"
    ).ap()

    with tile.TileContext(nc) as tc, ExitStack() as ctx:
        singles = ctx.enter_context(tc.tile_pool(name="singles", bufs=1))
        encp = ctx.enter_context(tc.tile_pool(name="encp", bufs=8))
        segp = ctx.enter_context(tc.tile_pool(name="segp", bufs=2))
        smalls = ctx.enter_context(tc.tile_pool(name="smalls", bufs=4))
        ps_seg = ctx.enter_context(tc.tile_pool(name="ps_seg", bufs=2, space="PSUM"))
        ps_tr = ctx.enter_context(tc.tile_pool(name="ps_tr", bufs=2, space="PSUM"))
        ps_pr = ctx.enter_context(tc.tile_pool(name="ps_pr", bufs=2, space="PSUM"))

        # ---- constants on the ACT ring; amat first (first matmul needs it,
        # and it's now one fully-contiguous 1 MiB transfer) ----
        a_sb = singles.tile([128, BPC, SCHUNKS, N_SENT], FP8)
        nc.scalar.dma_start(
            out=a_sb, in_=amat.rearrange("p (b k n) -> p b k n", k=SCHUNKS, n=N_SENT)
        )
        ident_sb = singles.tile([128, 128], BF16)
        nc.scalar.dma_start(out=ident_sb, in_=ident)
        lens_sb = singles.tile([N_SENT, BPC], F32)
        nc.scalar.dma_start(out=lens_sb, in_=lens)
        w_sb = singles.tile([128, DCH, D_OUT], BF16)
        nc.scalar.dma_start(out=w_sb, in_=wt.rearrange("p (j o) -> p j o", o=D_OUT))
        # b broadcast to [N_SENT, D_OUT] via stride-0 partition AP (SWDGE)
        b_bc = singles.tile([N_SENT, D_OUT], F32)
        bias_bcast = bass.AP(
            tensor=bias.tensor, offset=bias.offset,
            ap=[[0, N_SENT], [1, D_OUT]],
        )
        nc.gpsimd.dma_start(out=b_bc, in_=bias_bcast)

        # all-batch staging tiles for the softmax tail
        svs_all = singles.tile([N_SENT, BPC, D_OUT], F32)
        ssum_all = smalls.tile([N_SENT, BPC], F32, tag="ssum", bufs=1)
        ot_all = singles.tile([N_SENT, BPC, D_OUT], F32)

        n_pairs = SCHUNKS // 2
        pairs_per_dma = SS_PER_DMA // 2
        psums = {}

        def emit_tail(bi):
            """Per-batch tail: evict PSUM, transpose, project, softmax prep.

            Issued one batch late so it lands on the PE queue between the
            next batch's seg matmuls (inputs are already available then).
            """
            ps0, ps1 = psums.pop(bi)
            seg_sb = segp.tile([N_SENT, D_IN], BF16, tag="seg")
            nc.vector.tensor_copy(out=seg_sb[:, 0:512], in_=ps0)
            nc.scalar.copy(out=seg_sb[:, 512:1024], in_=ps1)

            # transpose seg [64, 1024] -> segT [128(d), 8(j), 64(n)] in bf16
            seg_t = segp.tile([128, DCH, N_SENT], BF16, tag="segT")
            for j in range(DCH):
                pt = ps_tr.tile([128, N_SENT], BF16, tag="pt")
                nc.tensor.transpose(
                    out=pt,
                    in_=seg_sb[:, j * 128 : (j + 1) * 128],
                    identity=ident_sb[0:N_SENT, 0:N_SENT],
                )
                if j % 2 == 0:
                    nc.vector.tensor_copy(out=seg_t[:, j, :], in_=pt)
                else:
                    nc.scalar.copy(out=seg_t[:, j, :], in_=pt)

            # projection: sv[n, o] = sum_d segT[d, n] * W[d, o]  (bf16)
            pp = ps_pr.tile([N_SENT, D_OUT], F32, tag="pp")
            for j in range(DCH):
                nc.tensor.matmul(
                    pp,
                    lhsT=seg_t[:, j, :],
                    rhs=w_sb[:, j, :],
                    start=(j == 0),
                    stop=(j == DCH - 1),
                )

            # sv = pp + len * b
            sv = smalls.tile([N_SENT, D_OUT], F32, tag="sv", bufs=2)
            nc.vector.scalar_tensor_tensor(
                out=sv,
                in0=b_bc,
                scalar=lens_sb[:, bi : bi + 1],
                in1=pp,
                op0=mybir.AluOpType.mult,
                op1=mybir.AluOpType.add,
            )
            # svs = sv - max(sv); ex = exp(svs); ssum = sum(ex)
            negmax = smalls.tile([N_SENT, 1], F32, tag="negmax", bufs=2)
            nc.vector.tensor_reduce(
                out=negmax, in_=sv, axis=mybir.AxisListType.X,
                op=mybir.AluOpType.max, negate=True,
            )
            nc.vector.tensor_scalar(
                out=svs_all[:, bi, :], in0=sv, scalar1=negmax,
                scalar2=None, op0=mybir.AluOpType.add,
            )
            ex = smalls.tile([N_SENT, D_OUT], F32, tag="ex", bufs=2)
            nc.scalar.activation(
                out=ex, in_=svs_all[:, bi, :],
                func=mybir.ActivationFunctionType.Exp,
            )
            nc.vector.tensor_reduce(
                out=ssum_all[:, bi : bi + 1], in_=ex, axis=mybir.AxisListType.X,
                op=mybir.AluOpType.add,
            )

        for bi in range(BPC):
            ps0 = ps_seg.tile([N_SENT, 512], F32, tag="ps0")
            ps1 = ps_seg.tile([N_SENT, 512], F32, tag="ps1")
            psums[bi] = (ps0, ps1)
            for kk in range(n_dma):
                et = encp.tile([128, SS_PER_DMA, D_IN], FP8, tag="enc")
                nc.sync.dma_start(
                    out=et,
                    in_=enc[bi, kk].rearrange("p (t d) -> p t d", d=D_IN),
                )
                if bi > 0 and kk == 1:
                    emit_tail(bi - 1)
                for u in range(pairs_per_dma):
                    pair = kk * pairs_per_dma + u
                    lhsT = a_sb[:, bi, 2 * pair : 2 * pair + 2, :]
                    for dh in range(2):
                        rhs = et[:, 2 * u : 2 * u + 2, dh * 512 : (dh + 1) * 512]
                        nc.tensor.matmul(
                            ps0 if dh == 0 else ps1,
                            lhsT=lhsT,
                            rhs=rhs,
                            start=(pair == 0),
                            stop=(pair == n_pairs - 1),
                            perf_mode=mybir.MatmulPerfMode.DoubleRow,
                        )
        emit_tail(BPC - 1)

        # ---- final: lse = ln(ssum), out = svs - lse, one output DMA ----
        lse_all = smalls.tile([N_SENT, BPC], F32, tag="lse", bufs=1)
        nc.scalar.activation(
            out=lse_all, in_=ssum_all, func=mybir.ActivationFunctionType.Ln
        )
        for bi in range(BPC):
            nc.vector.tensor_scalar(
                out=ot_all[:, bi, :], in0=svs_all[:, bi, :],
                scalar1=lse_all[:, bi : bi + 1], scalar2=None,
                op0=mybir.AluOpType.subtract,
            )
        nc.sync.dma_start(out=out.rearrange("b n o -> n b o"), in_=ot_all)

    nc.compile()
    return nc


_PROGRAM = None


def _get_program():
    global _PROGRAM
    if _PROGRAM is None:
        _PROGRAM = _build_program()
    return _PROGRAM


def _quantize_diffuse(enc):
    """fp8 e4m3 quantization with error diffusion along s (block=128).

    Within each contiguous 128-position block the rounding error of each
    element is carried into the next, so any in-block partial sum of the
    quantized values equals the exact partial sum plus at most ~one
    rounding step.  Segment sums then see only ~one step of error per
    block boundary crossed instead of sqrt(len) growth.
    """
    enc_r = enc.reshape(B, SCHUNKS, 128, D_IN)
    q = np.empty((B, SCHUNKS, 128, D_IN), dtype=_E4NP)
    carry = np.zeros((B, SCHUNKS, D_IN), dtype=np.float32)
    for i in range(128):
        t = enc_r[:, :, i, :] + carry
        qi = t.astype(_E4NP)
        q[:, :, i, :] = qi
        carry = t - qi.astype(np.float32)
    return q  # [B, k, p, D] with s = k*128 + p


def _host_prep(enc_output, W, b, cls_pos, last_sep):
    n_dma = SCHUNKS // SS_PER_DMA
    enc = np.asarray(enc_output, dtype=np.float32)
    q = _quantize_diffuse(enc)
    # [B, k, p, D] -> [B, n_dma, 128(p), SS_PER_DMA(t) * D]  with k = kk*SS+t
    enc8 = np.ascontiguousarray(
        q.reshape(B, n_dma, SS_PER_DMA, 128, D_IN)
        .transpose(0, 1, 3, 2, 4)
        .reshape(B, n_dma, 128, SS_PER_DMA * D_IN)
    )
    wf = np.asarray(W, dtype=np.float32)
    # [D_IN, D_OUT] -> bf16 [128(p), DCH(j) * D_OUT] with d = j*128+p
    wf = np.ascontiguousarray(
        wf.reshape(DCH, 128, D_OUT)
        .transpose(1, 0, 2)
        .reshape(128, DCH * D_OUT)
        .astype(_BF16NP)
    )
    bf = np.ascontiguousarray(np.asarray(b, dtype=np.float32))
    starts = np.asarray(cls_pos).astype(np.int64)                    # [B, N]
    lsep = np.asarray(last_sep).astype(np.int64)                     # [B]
    ends = np.concatenate([starts[:, 1:], (lsep + 1)[:, None]], axis=1)
    # torch semantics for the last segment: if end <= start, sum to seq end
    ends[:, -1] = np.where(ends[:, -1] > starts[:, -1], ends[:, -1], S)
    lens = (ends - starts).astype(np.float32)                        # [B, N]

    s = np.arange(S, dtype=np.int64)
    afull = (s[None, :, None] >= starts[:, None, :]) & (
        s[None, :, None] < ends[:, None, :]
    )                                                                # [B, S, N]
    return enc8, wf, bf, afull, lens


def _amat_tile(afull_c):
    """[BPC, S, N] bool -> contiguous [128(p), BPC, SCHUNKS, N] fp8 bytes."""
    a = (
        afull_c.reshape(BPC, SCHUNKS, 128, N_SENT)
        .transpose(2, 0, 1, 3)                       # [128, BPC, SCHUNKS, N]
        .reshape(128, BPC * SCHUNKS * N_SENT)
        .astype(np.float32)
        .astype(_E4NP)                               # 0.0 / 1.0 exact
    )
    return np.ascontiguousarray(a)


def kernel(enc_output, W, b, max_num_sent, cls_pos, last_sep, _trace=False):
    enc8, wf, bf, afull, lens = _host_prep(enc_output, W, b, cls_pos, last_sep)
    ident = np.eye(128, dtype=_BF16NP)

    nc = _get_program()
    in_maps = []
    for c in range(N_CORES):
        bsl = slice(c * BPC, (c + 1) * BPC)
        in_maps.append(
            {
                "enc": enc8[bsl],
                "w": wf,
                "bias": bf,
                "amat": _amat_tile(afull[bsl]),
                "lens": np.ascontiguousarray(lens[bsl].T),
                "ident": ident,
            }
        )
    res = bass_utils.run_bass_kernel_spmd(
        nc, in_maps, core_ids=list(range(N_CORES)), trace=_trace
    )
    out = np.concatenate(
        [res.results[c]["out"][None] for c in range(N_CORES)], axis=0
    ).reshape(B, N_SENT, D_OUT)
    if _trace:
        kernel._last_result = res
    return out.astype(np.float32)


# revision 9
# speedup vs baseline: 2.7491x; 1.0610x over previous
"""Trainium2 Bass kernel for nn_DialogActLabeller (segment_reduce).

Computes, for input enc_output [32, 4096, 1024], W [1024, 256], b [256],
cls_pos [32, 64], last_sep [32]:

    x = enc_output @ W + b                      # [B, S, 256]
    seg[b, n] = sum_{s in [start_n, end_n)} x[b, s, :]
    out = log_softmax(seg, axis=-1)             # [B, 64, 256]

Key algebraic restructure: the projection is linear, so segment-reduce
FIRST on enc_output (via a matmul with a 0/1 segment-indicator matrix A),
then project the tiny [64, 1024] per-batch result with W, and add
len_n * b for the bias.  This reads enc_output exactly once from HBM and
does ~1/32 of the naive FLOPs.

The kernel is HBM-bound (enc_output is 512 MiB), so enc is shipped as
fp8 e4m3.  Plain per-element rounding would make segment sums drift as
sqrt(len); instead the host quantizes with error diffusion along s
(within each 128-position block): the running rounding error is carried
into the next element, so partial sums telescope and each segment sum
carries only ~one rounding step of error regardless of length.

The segment-reduce matmul runs in fp8 DoubleRow mode (two stacked
128x64 weight sets -> full PE array, 2 contraction rows per cycle).
The per-batch tail (PSUM evict, transpose, projection, softmax prep) is
software-pipelined one batch behind the enc stream so the PE queue never
stalls at batch boundaries.

Sharding: pure data parallel, 4 batch rows per core across 8 cores
(W, b replicated), no cross-core communication.
"""

import numpy as np

import concourse.bacc as bacc
import concourse.bass as bass
import concourse.tile as tile
from concourse import mybir
from concourse import bass_utils
from contextlib import ExitStack

# Problem shapes (hardcoded per contract)
B, S, D_IN, D_OUT, N_SENT = 32, 4096, 1024, 256, 64
N_CORES = 8
BPC = B // N_CORES          # batches per core
SCHUNKS = S // 128          # 32 sequence chunks of 128
DCH = D_IN // 128           # 8 d_in chunks of 128
SS_PER_DMA = 8              # s-chunks per enc DMA (1 MiB fp8 transfers)

F32 = mybir.dt.float32
BF16 = mybir.dt.bfloat16
FP8 = mybir.dt.float8e4
_E4NP = mybir.dt.np(FP8)    # ml_dtypes.float8_e4m3
_BF16NP = mybir.dt.np(BF16)


def _build_program():
    nc = bacc.Bacc("TRN2", debug=False)

    n_dma = SCHUNKS // SS_PER_DMA
    enc = nc.dram_tensor(
        "enc", [BPC, n_dma, 128, SS_PER_DMA * D_IN], FP8, kind="ExternalInput"
    ).ap()
    # W host-pre-tiled to bf16 [128, DCH*D_OUT] with layout [p, j, o]
    wt = nc.dram_tensor("w", [128, DCH * D_OUT], BF16, kind="ExternalInput").ap()
    bias = nc.dram_tensor("bias", [D_OUT], F32, kind="ExternalInput").ap()
    # segment-indicator matrices in fp8, host-pre-tiled to the exact SBUF
    # layout [128(p), BPC, SCHUNKS, N_SENT] so the DMA is fully contiguous
    amat = nc.dram_tensor(
        "amat", [128, BPC * SCHUNKS * N_SENT], FP8, kind="ExternalInput"
    ).ap()
    lens = nc.dram_tensor("lens", [N_SENT, BPC], F32, kind="ExternalInput").ap()
    ident = nc.dram_tensor("ident", [128, 128], BF16, kind="ExternalInput").ap()
    out = nc.dram_tensor(
        "out", [BPC, N_SENT, D_OUT], F32, kind="ExternalOutput"
    ).ap()

    with tile.TileContext(nc) as tc, ExitStack() as ctx:
        singles = ctx.enter_context(tc.tile_pool(name="singles", bufs=1))
        encp = ctx.enter_context(tc.tile_pool(name="encp", bufs=8))
        segp = ctx.enter_context(tc.tile_pool(name="segp", bufs=2))
        smalls = ctx.enter_context(tc.tile_pool(name="smalls", bufs=4))
        ps_seg = ctx.enter_context(tc.tile_pool(name="ps_seg", bufs=2, space="PSUM"))
        ps_tr = ctx.enter_context(tc.tile_pool(name="ps_tr", bufs=2, space="PSUM"))
        ps_pr = ctx.enter_context(tc.tile_pool(name="ps_pr", bufs=2, space="PSUM"))

        # ---- constants on the ACT ring; amat first (first matmul needs it,
        # and it's now one fully-contiguous 1 MiB transfer) ----
        a_sb = singles.tile([128, BPC, SCHUNKS, N_SENT], FP8)
        nc.scalar.dma_start(
            out=a_sb, in_=amat.rearrange("p (b k n) -> p b k n", k=SCHUNKS, n=N_SENT)
        )
        ident_sb = singles.tile([128, 128], BF16)
        nc.scalar.dma_start(out=ident_sb, in_=ident)
        lens_sb = singles.tile([N_SENT, BPC], F32)
        nc.scalar.dma_start(out=lens_sb, in_=lens)
        w_sb = singles.tile([128, DCH, D_OUT], BF16)
        nc.scalar.dma_start(out=w_sb, in_=wt.rearrange("p (j o) -> p j o", o=D_OUT))
        # b broadcast to [N_SENT, D_OUT] via stride-0 partition AP (SWDGE)
        b_bc = singles.tile([N_SENT, D_OUT], F32)
        bias_bcast = bass.AP(
            tensor=bias.tensor, offset=bias.offset,
            ap=[[0, N_SENT], [1, D_OUT]],
        )
        nc.gpsimd.dma_start(out=b_bc, in_=bias_bcast)

        n_pairs = SCHUNKS // 2
        psums = {}

        def emit_tail(bi):
            """Per-batch tail: evict PSUM, transpose, project, softmax prep.

            Issued one batch late so it lands on the PE queue between the
            next batch's seg matmuls (inputs are already available then).
            """
            ps0, ps1 = psums.pop(bi)
            seg_sb = segp.tile([N_SENT, D_IN], BF16, tag="seg")
            nc.vector.tensor_copy(out=seg_sb[:, 0:512], in_=ps0)
            nc.scalar.copy(out=seg_sb[:, 512:1024], in_=ps1)

            # transpose seg [64, 1024] -> segT [128(d), 8(j), 64(n)] in bf16
            seg_t = segp.tile([128, DCH, N_SENT], BF16, tag="segT")
            for j in range(DCH):
                pt = ps_tr.tile([128, N_SENT], BF16, tag="pt")
                nc.tensor.transpose(
                    out=pt,
                    in_=seg_sb[:, j * 128 : (j + 1) * 128],
                    identity=ident_sb[0:N_SENT, 0:N_SENT],
                )
                if j % 2 == 0:
                    nc.vector.tensor_copy(out=seg_t[:, j, :], in_=pt)
                else:
                    nc.scalar.copy(out=seg_t[:, j, :], in_=pt)

            # projection: sv[n, o] = sum_d segT[d, n] * W[d, o]  (bf16)
            pp = ps_pr.tile([N_SENT, D_OUT], F32, tag="pp")
            for j in range(DCH):
                nc.tensor.matmul(
                    pp,
                    lhsT=seg_t[:, j, :],
                    rhs=w_sb[:, j, :],
                    start=(j == 0),
                    stop=(j == DCH - 1),
                )

            # sv = pp + len * b
            sv = smalls.tile([N_SENT, D_OUT], F32, tag="sv", bufs=2)
            nc.vector.scalar_tensor_tensor(
                out=sv,
                in0=b_bc,
                scalar=lens_sb[:, bi : bi + 1],
                in1=pp,
                op0=mybir.AluOpType.mult,
                op1=mybir.AluOpType.add,
            )
            # log_softmax: svs = sv - max(sv); out = svs - ln(sum(exp(svs)))
            negmax = smalls.tile([N_SENT, 1], F32, tag="negmax", bufs=2)
            nc.vector.tensor_reduce(
                out=negmax, in_=sv, axis=mybir.AxisListType.X,
                op=mybir.AluOpType.max, negate=True,
            )
            svs = smalls.tile([N_SENT, D_OUT], F32, tag="svs", bufs=2)
            nc.vector.tensor_scalar(
                out=svs, in0=sv, scalar1=negmax,
                scalar2=None, op0=mybir.AluOpType.add,
            )
            ex = smalls.tile([N_SENT, D_OUT], F32, tag="ex", bufs=2)
            nc.scalar.activation(
                out=ex, in_=svs, func=mybir.ActivationFunctionType.Exp,
            )
            ssum = smalls.tile([N_SENT, 1], F32, tag="ssum", bufs=2)
            nc.vector.tensor_reduce(
                out=ssum, in_=ex, axis=mybir.AxisListType.X,
                op=mybir.AluOpType.add,
            )
            lse = smalls.tile([N_SENT, 1], F32, tag="lse", bufs=2)
            nc.scalar.activation(
                out=lse, in_=ssum, func=mybir.ActivationFunctionType.Ln
            )
            ot = smalls.tile([N_SENT, D_OUT], F32, tag="ot", bufs=2)
            nc.vector.tensor_scalar(
                out=ot, in0=svs, scalar1=lse, scalar2=None,
                op0=mybir.AluOpType.subtract,
            )
            nc.scalar.dma_start(out=out[bi], in_=ot)

        # batch 0 starts with small DMA slices so the first matmul isn't
        # stuck behind a deep queue of round-robined 1 MiB transfers.
        plans = {0: [(0, 0, 2), (0, 2, 2), (0, 4, 4)]
                    + [(kk, 0, SS_PER_DMA) for kk in range(1, n_dma)]}
        for bi in range(1, BPC):
            plans[bi] = [(kk, 0, SS_PER_DMA) for kk in range(n_dma)]

        for bi in range(BPC):
            ps0 = ps_seg.tile([N_SENT, 512], F32, tag="ps0")
            ps1 = ps_seg.tile([N_SENT, 512], F32, tag="ps1")
            psums[bi] = (ps0, ps1)
            for ti, (kk, t0, nt) in enumerate(plans[bi]):
                et = encp.tile(
                    [128, nt, D_IN], FP8, tag=f"enc{nt}",
                    bufs=(8 if nt == SS_PER_DMA else 2),
                )
                nc.sync.dma_start(
                    out=et,
                    in_=enc[bi, kk][:, t0 * D_IN : (t0 + nt) * D_IN].rearrange(
                        "p (t d) -> p t d", d=D_IN
                    ),
                )
                if bi > 0 and ti == 1:
                    emit_tail(bi - 1)
                for u in range(nt // 2):
                    pair = (kk * SS_PER_DMA + t0) // 2 + u
                    lhsT = a_sb[:, bi, 2 * pair : 2 * pair + 2, :]
                    for dh in range(2):
                        rhs = et[:, 2 * u : 2 * u + 2, dh * 512 : (dh + 1) * 512]
                        nc.tensor.matmul(
                            ps0 if dh == 0 else ps1,
                            lhsT=lhsT,
                            rhs=rhs,
                            start=(pair == 0),
                            stop=(pair == n_pairs - 1),
                            perf_mode=mybir.MatmulPerfMode.DoubleRow,
                        )
        emit_tail(BPC - 1)

    nc.compile()
    return nc


_PROGRAM = None


def _get_program():
    global _PROGRAM
    if _PROGRAM is None:
        _PROGRAM = _build_program()
    return _PROGRAM


def _quantize_diffuse(enc):
    """fp8 e4m3 quantization with error diffusion along s (block=128).

    Within each contiguous 128-position block the rounding error of each
    element is carried into the next, so any in-block partial sum of the
    quantized values equals the exact partial sum plus at most ~one
    rounding step.  Segment sums then see only ~one step of error per
    block boundary crossed instead of sqrt(len) growth.
    """
    enc_r = enc.reshape(B, SCHUNKS, 128, D_IN)
    q = np.empty((B, SCHUNKS, 128, D_IN), dtype=_E4NP)
    carry = np.zeros((B, SCHUNKS, D_IN), dtype=np.float32)
    for i in range(128):
        t = enc_r[:, :, i, :] + carry
        qi = t.astype(_E4NP)
        q[:, :, i, :] = qi
        carry = t - qi.astype(np.float32)
    return q  # [B, k, p, D] with s = k*128 + p


def _host_prep(enc_output, W, b, cls_pos, last_sep):
    n_dma = SCHUNKS // SS_PER_DMA
    enc = np.asarray(enc_output, dtype=np.float32)
    q = _quantize_diffuse(enc)
    # [B, k, p, D] -> [B, n_dma, 128(p), SS_PER_DMA(t) * D]  with k = kk*SS+t
    enc8 = np.ascontiguousarray(
        q.reshape(B, n_dma, SS_PER_DMA, 128, D_IN)
        .transpose(0, 1, 3, 2, 4)
        .reshape(B, n_dma, 128, SS_PER_DMA * D_IN)
    )
    wf = np.asarray(W, dtype=np.float32)
    # [D_IN, D_OUT] -> bf16 [128(p), DCH(j) * D_OUT] with d = j*128+p
    wf = np.ascontiguousarray(
        wf.reshape(DCH, 128, D_OUT)
        .transpose(1, 0, 2)
        .reshape(128, DCH * D_OUT)
        .astype(_BF16NP)
    )
    bf = np.ascontiguousarray(np.asarray(b, dtype=np.float32))
    starts = np.asarray(cls_pos).astype(np.int64)                    # [B, N]
    lsep = np.asarray(last_sep).astype(np.int64)                     # [B]
    ends = np.concatenate([starts[:, 1:], (lsep + 1)[:, None]], axis=1)
    # torch semantics for the last segment: if end <= start, sum to seq end
    ends[:, -1] = np.where(ends[:, -1] > starts[:, -1], ends[:, -1], S)
    lens = (ends - starts).astype(np.float32)                        # [B, N]

    s = np.arange(S, dtype=np.int64)
    afull = (s[None, :, None] >= starts[:, None, :]) & (
        s[None, :, None] < ends[:, None, :]
    )                                                                # [B, S, N]
    return enc8, wf, bf, afull, lens


def _amat_tile(afull_c):
    """[BPC, S, N] bool -> contiguous [128(p), BPC, SCHUNKS, N] fp8 bytes."""
    a = (
        afull_c.reshape(BPC, SCHUNKS, 128, N_SENT)
        .transpose(2, 0, 1, 3)                       # [128, BPC, SCHUNKS, N]
        .reshape(128, BPC * SCHUNKS * N_SENT)
        .astype(np.float32)
        .astype(_E4NP)                               # 0.0 / 1.0 exact
    )
    return np.ascontiguousarray(a)


def kernel(enc_output, W, b, max_num_sent, cls_pos, last_sep, _trace=False):
    enc8, wf, bf, afull, lens = _host_prep(enc_output, W, b, cls_pos, last_sep)
    ident = np.eye(128, dtype=_BF16NP)

    nc = _get_program()
    in_maps = []
    for c in range(N_CORES):
        bsl = slice(c * BPC, (c + 1) * BPC)
        in_maps.append(
            {
                "enc": enc8[bsl],
                "w": wf,
                "bias": bf,
                "amat": _amat_tile(afull[bsl]),
                "lens": np.ascontiguousarray(lens[bsl].T),
                "ident": ident,
            }
        )
    res = bass_utils.run_bass_kernel_spmd(
        nc, in_maps, core_ids=list(range(N_CORES)), trace=_trace
    )
    out = np.concatenate(
        [res.results[c]["out"][None] for c in range(N_CORES)], axis=0
    ).reshape(B, N_SENT, D_OUT)
    if _trace:
        kernel._last_result = res
    return out.astype(np.float32)
